# revision 3
# baseline (speedup 1.0000x reference)
"""Trainium2 Bass kernel for the edge-MLP decoder (gnn_message_passing).

Computes, for every edge (s, d):
    out = sigmoid(relu(relu([z[s]; z[d]] @ W1 + b1) @ W2 + b2) @ W3 + b3)

Strategy (8 NeuronCores):
  * Edges are sharded across cores by src node range (8 ranges of 12500).
    Within a core, edges are bucketed by dst range (8 classes of 12500).
  * z is pre-tiled host-side into 8 per-range SBUF "gather images"
    ([128 partitions = token%128, rank = token//128, 256 B per row], fp16).
    The core's src image stays resident in SBUF; dst images are streamed
    HBM->SBUF per class with double buffering (3.2 MB sequential loads).
  * Edge endpoints are fetched with SBUF-source SWDGE dma_gather in
    transpose mode: each gathered row lands as a column of the [K=128, E]
    feature-major tile the PE array needs. Gathers are spread over 4 SWDGE
    queues - descriptor generation runs on a different GPSIMD Q7 core pair
    per queue, which is the gather throughput limit (measured ~4x speedup
    vs one queue, ~1.2 ns/row).
  * All matmuls run in fp16 (fp32 PSUM accumulation). relu/bias fusions on
    ACT and DVE, sigmoid on ACT. The W3 logit matmul uses a rotated
    [128, 255] weight image so 128 blocks share one PSUM bank.
"""

import numpy as np
from contextlib import ExitStack

import concourse.bass as bass
import concourse.tile as tile
from concourse import bacc, mybir
from concourse.bass_utils import run_bass_kernel_spmd

# ---- static problem geometry (nn_Decoder_81819126989051) ----
N_NODES = 100000
D = 128                    # node feature dim
N_CORES = 8
R = 12500                  # nodes per range (src shard / dst class)
TPR = 128                  # gather-image tokens per rank
NRANK = (R + TPR - 1) // TPR   # 98 ranks -> 12544 token slots
NCLS = 8                   # dst classes per core
BLK = 512                  # edges per matmul block (PSUM bank width)
CHUNK = 4096               # edges per dma_gather call
NQ = 1                     # SWDGE queues

# Static per-class slot counts (max over cores for the key-0 dataset,
# rounded to BLK; overflowing edges fall back to the host path).
NI = [15872, 15872, 15872, 15872, 15872, 15872, 15872, 16384]
B_N = [n // BLK for n in NI]          # blocks per class
B_OFF = np.concatenate([[0], np.cumsum(B_N)]).astype(int)
B_USED = int(B_OFF[-1])               # 249
CAP = B_USED * BLK                    # slots per core
OUT_CH = (B_USED + 127) // 128        # output staging column chunks

F16 = mybir.dt.float16
F32 = mybir.dt.float32
I16 = mybir.dt.int16
AF = mybir.ActivationFunctionType
ALU = mybir.AluOpType

_prog_cache = None


def _class_chunks(ni):
    """Split a class's ni slots into dma_gather chunk sizes."""
    out = []
    while ni > 0:
        out.append(min(CHUNK, ni))
        ni -= out[-1]
    return out


def _build_program(do_gather=True, do_compute=True):
    nc = bacc.Bacc(
        "TRN2", target_bir_lowering=False, debug=False, num_devices=N_CORES,
        dynamic_dma_scratch_size=49152, num_swdge_queues=NQ,
    )

    zs_d = nc.declare_dram_parameter("zs", [128, NRANK * D], F16, isOutput=False)
    zd_d = [
        nc.declare_dram_parameter(f"zd{k}", [128, NRANK * D], F16, isOutput=False)
        for k in range(NCLS)
    ]
    sidx_d = nc.declare_dram_parameter("sidx", [128, CAP // 16], I16, isOutput=False)
    didx_d = nc.declare_dram_parameter("didx", [128, CAP // 16], I16, isOutput=False)
    w1s_d = nc.declare_dram_parameter("w1s", [128, 256], F16, isOutput=False)
    w1d_d = nc.declare_dram_parameter("w1d", [128, 256], F16, isOutput=False)
    w2a_d = nc.declare_dram_parameter("w2a", [128, 128], F16, isOutput=False)
    w2b_d = nc.declare_dram_parameter("w2b", [128, 128], F16, isOutput=False)
    # w3v[:, 127] = W3; other columns zero.  lhsT slice [127-p : 255-p] puts
    # W3 in output-partition p of the shared logit PSUM bank.
    w3v_d = nc.declare_dram_parameter("w3v", [128, 255], F16, isOutput=False)
    b1a_d = nc.declare_dram_parameter("b1a", [128, 1], F32, isOutput=False)
    b1b_d = nc.declare_dram_parameter("b1b", [128, 1], F32, isOutput=False)
    b2_d = nc.declare_dram_parameter("b2", [128, 1], F32, isOutput=False)
    b3_d = nc.declare_dram_parameter("b3", [128, 1], F32, isOutput=False)
    out_d = nc.declare_dram_parameter("out", [B_USED, BLK], F32, isOutput=True)

    with tile.TileContext(nc) as tc, ExitStack() as ctx:
        const = ctx.enter_context(tc.tile_pool(name="const", bufs=1))

        def load_const(dram, shape, dtype):
            t = const.tile(shape, dtype, tag=dram.name + "_sb")
            nc.sync.dma_start(out=t[:], in_=dram[:])
            return t

        tw1s = load_const(w1s_d, [128, 256], F16)
        tw1d = load_const(w1d_d, [128, 256], F16)
        tw2a = load_const(w2a_d, [128, 128], F16)
        tw2b = load_const(w2b_d, [128, 128], F16)
        tw3v = load_const(w3v_d, [128, 255], F16)
        tb1a = load_const(b1a_d, [128, 1], F32)
        tb1b = load_const(b1b_d, [128, 1], F32)
        tb2 = load_const(b2_d, [128, 1], F32)
        tb3 = load_const(b3_d, [128, 1], F32)
        tzs = load_const(zs_d, [128, NRANK * D], F16)
        tout = const.tile([128, OUT_CH * BLK], F32, tag="out_sb")

        zpool = ctx.enter_context(tc.tile_pool(name="zdst", bufs=2))
        ipool = ctx.enter_context(tc.tile_pool(name="idx", bufs=4))
        gpool = ctx.enter_context(tc.tile_pool(name="gath", bufs=6))
        h1pool = ctx.enter_context(tc.tile_pool(name="h1s", bufs=4))
        h2pool = ctx.enter_context(tc.tile_pool(name="h2s", bufs=3))
        ph1 = ctx.enter_context(tc.tile_pool(name="ph1", bufs=4, space="PSUM"))
        ph2 = ctx.enter_context(tc.tile_pool(name="ph2", bufs=2, space="PSUM"))
        plg = ctx.enter_context(tc.tile_pool(name="plg", bufs=2, space="PSUM"))

        lg = None
        qn = 0
        for k in range(NCLS):
            tzd = zpool.tile([128, NRANK * D], F16, tag="zdst")
            nc.sync.dma_start(out=tzd[:], in_=zd_d[k][:])
            ts = ipool.tile([128, NI[k] // 16], I16, tag="sidx")
            td = ipool.tile([128, NI[k] // 16], I16, tag="didx")
            off = int(B_OFF[k]) * BLK // 16
            nc.sync.dma_start(out=ts[:], in_=sidx_d[:, off:off + NI[k] // 16])
            nc.sync.dma_start(out=td[:], in_=didx_d[:, off:off + NI[k] // 16])

            pos = 0
            for n in _class_chunks(NI[k]):
                sg = gpool.tile([128, 1, CHUNK], F16, tag="gath")
                dg = gpool.tile([128, 1, CHUNK], F16, tag="gath")
                if do_gather:
                    nc.gpsimd.dma_gather(
                        sg[:, :, 0:n], tzs[:],
                        ts[:, pos // 16:(pos + n) // 16], n, n, D,
                        transpose=True, single_packet=False, queue_num=qn,
                        sbuf_tokens_per_rank=TPR, sbuf_free_dim_per_rank=D * 2,
                        sbuf_free_dim_pad_per_rank=0, sbuf_byte_offset=0,
                    )
                    qn = (qn + 1) % NQ
                    nc.gpsimd.dma_gather(
                        dg[:, :, 0:n], tzd[:],
                        td[:, pos // 16:(pos + n) // 16], n, n, D,
                        transpose=True, single_packet=False, queue_num=qn,
                        sbuf_tokens_per_rank=TPR, sbuf_free_dim_per_rank=D * 2,
                        sbuf_free_dim_pad_per_rank=0, sbuf_byte_offset=0,
                    )
                    qn = (qn + 1) % NQ
                elif do_compute:
                    nc.gpsimd.memset(sg[:], 0.0)
                    nc.gpsimd.memset(dg[:], 0.0)
                if do_compute:
                    for j in range(n // BLK):
                        b = int(B_OFF[k]) + (pos + j * BLK) // BLK
                        sT = sg[:, 0, j * BLK:(j + 1) * BLK]
                        dT = dg[:, 0, j * BLK:(j + 1) * BLK]

                        h1a = ph1.tile([128, BLK], F32, tag="ph1")
                        nc.tensor.matmul(out=h1a[:], lhsT=tw1s[:, 0:128], rhs=sT, start=True, stop=False)
                        nc.tensor.matmul(out=h1a[:], lhsT=tw1d[:, 0:128], rhs=dT, start=False, stop=True)
                        h1b = ph1.tile([128, BLK], F32, tag="ph1")
                        nc.tensor.matmul(out=h1b[:], lhsT=tw1s[:, 128:256], rhs=sT, start=True, stop=False)
                        nc.tensor.matmul(out=h1b[:], lhsT=tw1d[:, 128:256], rhs=dT, start=False, stop=True)

                        h1sa = h1pool.tile([128, BLK], F16, tag="h1s")
                        nc.scalar.activation(h1sa[:], h1a[:], AF.Relu, bias=tb1a[:])
                        h1sb = h1pool.tile([128, BLK], F16, tag="h1s")
                        nc.vector.tensor_scalar(
                            out=h1sb[:], in0=h1b[:], scalar1=tb1b[:], scalar2=0.0,
                            op0=ALU.add, op1=ALU.max,
                        )

                        h2p = ph2.tile([128, BLK], F32, tag="ph2")
                        nc.tensor.matmul(out=h2p[:], lhsT=tw2a[:], rhs=h1sa[:], start=True, stop=False)
                        nc.tensor.matmul(out=h2p[:], lhsT=tw2b[:], rhs=h1sb[:], start=False, stop=True)
                        h2s = h2pool.tile([128, BLK], F16, tag="h2s")
                        nc.vector.tensor_scalar(
                            out=h2s[:], in0=h2p[:], scalar1=tb2[:], scalar2=0.0,
                            op0=ALU.add, op1=ALU.max,
                        )

                        p, ch = b % 128, b // 128
                        if p == 0:
                            lg = plg.tile([128, BLK], F32, tag="plg")
                        nc.tensor.matmul(
                            out=lg[:], lhsT=tw3v[:, 127 - p:255 - p], rhs=h2s[:],
                            start=(p == 0), stop=(p == 127 or b == B_USED - 1),
                            skip_group_check=True,
                        )
                        if p == 127 or b == B_USED - 1:
                            nc.scalar.activation(
                                tout[:, ch * BLK:(ch + 1) * BLK], lg[:], AF.Sigmoid,
                                bias=tb3[:],
                            )
                pos += n

        if do_compute:
            for ch in range(OUT_CH):
                rows = min(128, B_USED - ch * 128)
                nc.sync.dma_start(
                    out=out_d[ch * 128: ch * 128 + rows, :],
                    in_=tout[0:rows, ch * BLK:(ch + 1) * BLK],
                )

    nc.compile()
    return nc


def _w3v(W3):
    v = np.zeros((128, 255), np.float16)
    v[:, 127] = W3.astype(np.float16).reshape(-1)
    return v


def _z_image(z16_range):
    """[<=R, 128] fp16 -> [128, NRANK*128] gather-image layout."""
    buf = np.zeros((NRANK * TPR, D), np.float16)
    buf[:z16_range.shape[0]] = z16_range
    return np.ascontiguousarray(
        buf.reshape(NRANK, TPR, D).transpose(1, 0, 2).reshape(TPR, NRANK * D))


def _wrap_idx(arr):
    """[n] int16 -> [128, n//16] wrapped (16-partition pattern, replicated)."""
    t = arr.reshape(-1, 16).T  # [16, n//16]
    return np.tile(t, (8, 1))


def _mlp_ref_f32(zs, zd, W1, b1, W2, b2, W3, b3):
    ef = np.concatenate([zs, zd], axis=1)
    h = np.maximum(ef @ W1 + b1, 0.0)
    h = np.maximum(h @ W2 + b2, 0.0)
    o = h @ W3 + b3
    return 1.0 / (1.0 + np.exp(-o[:, 0]))


def _pack_inputs(z, ei, W1, b1, W2, b2, W3, b3):
    """Shard edges by src range, bucket by dst range; build gather images."""
    z16 = z.astype(np.float16)
    zimg = [_z_image(z16[r * R:(r + 1) * R]) for r in range(NCLS)]
    w_common = {
        "w1s": np.ascontiguousarray(W1[:128].astype(np.float16)),
        "w1d": np.ascontiguousarray(W1[128:].astype(np.float16)),
        "w2a": np.ascontiguousarray(W2[:128].astype(np.float16)),
        "w2b": np.ascontiguousarray(W2[128:].astype(np.float16)),
        "w3v": _w3v(W3),
        "b1a": np.ascontiguousarray(b1[:128].reshape(128, 1)).astype(np.float32),
        "b1b": np.ascontiguousarray(b1[128:].reshape(128, 1)).astype(np.float32),
        "b2": np.ascontiguousarray(b2.reshape(128, 1)).astype(np.float32),
        "b3": np.full((128, 1), np.float32(b3.reshape(-1)[0])),
    }
    for k in range(NCLS):
        w_common[f"zd{k}"] = zimg[k]

    src_all, dst_all = ei[0], ei[1]
    core_of = src_all // R
    in_maps, metas = [], []
    for c in range(N_CORES):
        epos = np.nonzero(core_of == c)[0]
        src = src_all[epos]
        dst = dst_all[epos]
        cls = dst // R
        order = np.argsort(cls, kind="stable")
        counts = np.bincount(cls, minlength=NCLS)
        starts = np.zeros(NCLS + 1, np.int64)
        np.cumsum(counts, out=starts[1:])
        sidx = np.zeros(CAP, np.int16)
        didx = np.zeros(CAP, np.int16)
        kept, overflow = [], []
        for k in range(NCLS):
            seg = order[starts[k]:starts[k + 1]]
            if len(seg) > NI[k]:
                overflow.append(epos[seg[NI[k]:]])
                seg = seg[:NI[k]]
            n = len(seg)
            o = int(B_OFF[k]) * BLK
            sidx[o:o + n] = (src[seg] - c * R).astype(np.int16)
            didx[o:o + n] = (dst[seg] - k * R).astype(np.int16)
            kept.append(epos[seg])
        metas.append((kept, overflow))
        in_maps.append({
            **w_common,
            "zs": zimg[c],
            "sidx": np.ascontiguousarray(_wrap_idx(sidx)),
            "didx": np.ascontiguousarray(_wrap_idx(didx)),
        })
    return in_maps, metas


def _unpack_outputs(core_outs, metas, ei, z, W1, b1, W2, b2, W3, b3):
    E = ei.shape[1]
    out = np.empty(E, dtype=np.float32)
    for c in range(N_CORES):
        flat = np.asarray(core_outs[c], dtype=np.float32).reshape(CAP)
        kept, overflow = metas[c]
        for k in range(NCLS):
            pos = kept[k]
            o = int(B_OFF[k]) * BLK
            out[pos] = flat[o:o + len(pos)]
        for pos in overflow:
            # Host fallback for edges beyond static class capacity
            # (does not trigger for the benchmark dataset).
            out[pos] = _mlp_ref_f32(
                z[ei[0, pos]], z[ei[1, pos]], W1, b1, W2, b2, W3, b3)
    return out


def _run(z, edge_index, W1, b1, W2, b2, W3, b3, **spmd_kwargs):
    global _prog_cache
    z = np.asarray(z, dtype=np.float32)
    W1 = np.asarray(W1, dtype=np.float32)
    b1 = np.asarray(b1, dtype=np.float32)
    W2 = np.asarray(W2, dtype=np.float32)
    b2 = np.asarray(b2, dtype=np.float32)
    W3 = np.asarray(W3, dtype=np.float32)
    b3 = np.asarray(b3, dtype=np.float32)
    ei = np.asarray(edge_index).astype(np.int64)
    assert z.shape == (N_NODES, D) and ei.shape[0] == 2

    if _prog_cache is None:
        _prog_cache = _build_program()
    nc = _prog_cache

    in_maps, metas = _pack_inputs(z, ei, W1, b1, W2, b2, W3, b3)
    br = run_bass_kernel_spmd(nc, in_maps, list(range(N_CORES)), **spmd_kwargs)
    core_outs = [br.results[c]["out"] for c in range(N_CORES)]
    out = _unpack_outputs(core_outs, metas, ei, z, W1, b1, W2, b2, W3, b3)
    return out, br


def kernel(z, edge_index, W1, b1, W2, b2, W3, b3):
    out, _ = _run(z, edge_index, W1, b1, W2, b2, W3, b3)
    return out


# revision 5
# speedup vs baseline: 1.0085x; 1.0085x over previous
"""Trainium2 Bass kernel for the edge-MLP decoder (gnn_message_passing).

Computes, for every edge (s, d):
    out = sigmoid(relu(relu([z[s]; z[d]] @ W1 + b1) @ W2 + b2) @ W3 + b3)

Strategy (8 NeuronCores):
  * Edges are sharded across cores by src node range (8 ranges of 12500).
    Within a core, edges are bucketed by dst range (8 classes of 12500).
  * z is pre-tiled host-side into 8 per-range SBUF "gather images"
    ([128 partitions = token%128, rank = token//128, 256 B per row], fp16).
    The core's src image stays resident in SBUF; dst images are streamed
    HBM->SBUF per class with double buffering (3.2 MB sequential loads).
  * Edge endpoints are fetched with SBUF-source SWDGE dma_gather in
    transpose mode: each gathered row lands as a column of the [K=128, E]
    feature-major tile the PE array needs. Gathers are spread over 4 SWDGE
    queues - descriptor generation runs on a different GPSIMD Q7 core pair
    per queue, which is the gather throughput limit (measured ~4x speedup
    vs one queue, ~1.2 ns/row).
  * All matmuls run in fp16 (fp32 PSUM accumulation). relu/bias fusions on
    ACT and DVE, sigmoid on ACT. The W3 logit matmul uses a rotated
    [128, 255] weight image so 128 blocks share one PSUM bank.
"""

import numpy as np
from contextlib import ExitStack

import concourse.bass as bass
import concourse.tile as tile
from concourse import bacc, mybir
from concourse.bass_utils import run_bass_kernel_spmd

# ---- static problem geometry (nn_Decoder_81819126989051) ----
N_NODES = 100000
D = 128                    # node feature dim
N_CORES = 8
R = 12500                  # nodes per range (src shard / dst class)
TPR = 128                  # gather-image tokens per rank
NRANK = (R + TPR - 1) // TPR   # 98 ranks -> 12544 token slots
NCLS = 8                   # dst classes per core
BLK = 512                  # edges per matmul block (PSUM bank width)
CHUNK = 8192               # edges per dma_gather call
NQ = 1                     # SWDGE queues

# Static per-class slot counts (max over cores for the key-0 dataset,
# rounded to BLK; overflowing edges fall back to the host path).
NI = [15872, 15872, 15872, 15872, 15872, 15872, 15872, 16384]
B_N = [n // BLK for n in NI]          # blocks per class
B_OFF = np.concatenate([[0], np.cumsum(B_N)]).astype(int)
B_USED = int(B_OFF[-1])               # 249
CAP = B_USED * BLK                    # slots per core
OUT_CH = (B_USED + 127) // 128        # output staging column chunks

F16 = mybir.dt.float16
F32 = mybir.dt.float32
I16 = mybir.dt.int16
AF = mybir.ActivationFunctionType
ALU = mybir.AluOpType

_prog_cache = None


def _class_chunks(ni):
    """Split a class's ni slots into dma_gather chunk sizes."""
    out = []
    while ni > 0:
        out.append(min(CHUNK, ni))
        ni -= out[-1]
    return out


def _build_program(do_gather=True, do_compute=True):
    nc = bacc.Bacc(
        "TRN2", target_bir_lowering=False, debug=False, num_devices=N_CORES,
        dynamic_dma_scratch_size=65536, num_swdge_queues=NQ,
    )

    zs_d = nc.declare_dram_parameter("zs", [128, NRANK * D], F16, isOutput=False)
    zd_d = [
        nc.declare_dram_parameter(f"zd{k}", [128, NRANK * D], F16, isOutput=False)
        for k in range(NCLS)
    ]
    sidx_d = nc.declare_dram_parameter("sidx", [128, CAP // 16], I16, isOutput=False)
    didx_d = nc.declare_dram_parameter("didx", [128, CAP // 16], I16, isOutput=False)
    w1s_d = nc.declare_dram_parameter("w1s", [128, 256], F16, isOutput=False)
    w1d_d = nc.declare_dram_parameter("w1d", [128, 256], F16, isOutput=False)
    w2a_d = nc.declare_dram_parameter("w2a", [128, 128], F16, isOutput=False)
    w2b_d = nc.declare_dram_parameter("w2b", [128, 128], F16, isOutput=False)
    # w3v[:, 127] = W3; other columns zero.  lhsT slice [127-p : 255-p] puts
    # W3 in output-partition p of the shared logit PSUM bank.
    w3v_d = nc.declare_dram_parameter("w3v", [128, 255], F16, isOutput=False)
    b1a_d = nc.declare_dram_parameter("b1a", [128, 1], F32, isOutput=False)
    b1b_d = nc.declare_dram_parameter("b1b", [128, 1], F32, isOutput=False)
    b2_d = nc.declare_dram_parameter("b2", [128, 1], F32, isOutput=False)
    b3_d = nc.declare_dram_parameter("b3", [128, 1], F32, isOutput=False)
    out_d = nc.declare_dram_parameter("out", [B_USED, BLK], F32, isOutput=True)

    with tile.TileContext(nc) as tc, ExitStack() as ctx:
        const = ctx.enter_context(tc.tile_pool(name="const", bufs=1))

        def load_const(dram, shape, dtype):
            t = const.tile(shape, dtype, tag=dram.name + "_sb")
            nc.sync.dma_start(out=t[:], in_=dram[:])
            return t

        tw1s = load_const(w1s_d, [128, 256], F16)
        tw1d = load_const(w1d_d, [128, 256], F16)
        tw2a = load_const(w2a_d, [128, 128], F16)
        tw2b = load_const(w2b_d, [128, 128], F16)
        tw3v = load_const(w3v_d, [128, 255], F16)
        tb1a = load_const(b1a_d, [128, 1], F32)
        tb1b = load_const(b1b_d, [128, 1], F32)
        tb2 = load_const(b2_d, [128, 1], F32)
        tb3 = load_const(b3_d, [128, 1], F32)
        tzs = load_const(zs_d, [128, NRANK * D], F16)
        tout = const.tile([128, OUT_CH * BLK], F32, tag="out_sb")

        zpool = ctx.enter_context(tc.tile_pool(name="zdst", bufs=2))
        ipool = ctx.enter_context(tc.tile_pool(name="idx", bufs=2))
        gpool = ctx.enter_context(tc.tile_pool(name="gath", bufs=4))
        h1pool = ctx.enter_context(tc.tile_pool(name="h1s", bufs=4))
        h2pool = ctx.enter_context(tc.tile_pool(name="h2s", bufs=3))
        ph1 = ctx.enter_context(tc.tile_pool(name="ph1", bufs=4, space="PSUM"))
        ph2 = ctx.enter_context(tc.tile_pool(name="ph2", bufs=2, space="PSUM"))
        plg = ctx.enter_context(tc.tile_pool(name="plg", bufs=2, space="PSUM"))

        lg = None
        qn = 0
        for k in range(NCLS):
            tzd = zpool.tile([128, NRANK * D], F16, tag="zdst")
            nc.sync.dma_start(out=tzd[:], in_=zd_d[k][:])
            ts = ipool.tile([128, NI[k] // 16], I16, tag="sidx")
            td = ipool.tile([128, NI[k] // 16], I16, tag="didx")
            off = int(B_OFF[k]) * BLK // 16
            nc.sync.dma_start(out=ts[:], in_=sidx_d[:, off:off + NI[k] // 16])
            nc.sync.dma_start(out=td[:], in_=didx_d[:, off:off + NI[k] // 16])

            pos = 0
            for n in _class_chunks(NI[k]):
                sg = gpool.tile([128, 1, CHUNK], F16, tag="gath")
                dg = gpool.tile([128, 1, CHUNK], F16, tag="gath")
                if do_gather:
                    nc.gpsimd.dma_gather(
                        sg[:, :, 0:n], tzs[:],
                        ts[:, pos // 16:(pos + n) // 16], n, n, D,
                        transpose=True, single_packet=False, queue_num=qn,
                        sbuf_tokens_per_rank=TPR, sbuf_free_dim_per_rank=D * 2,
                        sbuf_free_dim_pad_per_rank=0, sbuf_byte_offset=0,
                    )
                    qn = (qn + 1) % NQ
                    nc.gpsimd.dma_gather(
                        dg[:, :, 0:n], tzd[:],
                        td[:, pos // 16:(pos + n) // 16], n, n, D,
                        transpose=True, single_packet=False, queue_num=qn,
                        sbuf_tokens_per_rank=TPR, sbuf_free_dim_per_rank=D * 2,
                        sbuf_free_dim_pad_per_rank=0, sbuf_byte_offset=0,
                    )
                    qn = (qn + 1) % NQ
                elif do_compute:
                    nc.gpsimd.memset(sg[:], 0.0)
                    nc.gpsimd.memset(dg[:], 0.0)
                if do_compute:
                    for j in range(n // BLK):
                        b = int(B_OFF[k]) + (pos + j * BLK) // BLK
                        sT = sg[:, 0, j * BLK:(j + 1) * BLK]
                        dT = dg[:, 0, j * BLK:(j + 1) * BLK]

                        h1a = ph1.tile([128, BLK], F32, tag="ph1")
                        nc.tensor.matmul(out=h1a[:], lhsT=tw1s[:, 0:128], rhs=sT, start=True, stop=False)
                        nc.tensor.matmul(out=h1a[:], lhsT=tw1d[:, 0:128], rhs=dT, start=False, stop=True)
                        h1b = ph1.tile([128, BLK], F32, tag="ph1")
                        nc.tensor.matmul(out=h1b[:], lhsT=tw1s[:, 128:256], rhs=sT, start=True, stop=False)
                        nc.tensor.matmul(out=h1b[:], lhsT=tw1d[:, 128:256], rhs=dT, start=False, stop=True)

                        h1sa = h1pool.tile([128, BLK], F16, tag="h1s")
                        nc.scalar.activation(h1sa[:], h1a[:], AF.Relu, bias=tb1a[:])
                        h1sb = h1pool.tile([128, BLK], F16, tag="h1s")
                        nc.vector.tensor_scalar(
                            out=h1sb[:], in0=h1b[:], scalar1=tb1b[:], scalar2=0.0,
                            op0=ALU.add, op1=ALU.max,
                        )

                        h2p = ph2.tile([128, BLK], F32, tag="ph2")
                        nc.tensor.matmul(out=h2p[:], lhsT=tw2a[:], rhs=h1sa[:], start=True, stop=False)
                        nc.tensor.matmul(out=h2p[:], lhsT=tw2b[:], rhs=h1sb[:], start=False, stop=True)
                        h2s = h2pool.tile([128, BLK], F16, tag="h2s")
                        nc.vector.tensor_scalar(
                            out=h2s[:], in0=h2p[:], scalar1=tb2[:], scalar2=0.0,
                            op0=ALU.add, op1=ALU.max,
                        )

                        p, ch = b % 128, b // 128
                        if p == 0:
                            lg = plg.tile([128, BLK], F32, tag="plg")
                        nc.tensor.matmul(
                            out=lg[:], lhsT=tw3v[:, 127 - p:255 - p], rhs=h2s[:],
                            start=(p == 0), stop=(p == 127 or b == B_USED - 1),
                            skip_group_check=True,
                        )
                        if p == 127 or b == B_USED - 1:
                            nc.scalar.activation(
                                tout[:, ch * BLK:(ch + 1) * BLK], lg[:], AF.Sigmoid,
                                bias=tb3[:],
                            )
                pos += n

        if do_compute:
            for ch in range(OUT_CH):
                rows = min(128, B_USED - ch * 128)
                nc.sync.dma_start(
                    out=out_d[ch * 128: ch * 128 + rows, :],
                    in_=tout[0:rows, ch * BLK:(ch + 1) * BLK],
                )

    nc.compile()
    return nc


def _w3v(W3):
    v = np.zeros((128, 255), np.float16)
    v[:, 127] = W3.astype(np.float16).reshape(-1)
    return v


def _z_image(z16_range):
    """[<=R, 128] fp16 -> [128, NRANK*128] gather-image layout."""
    buf = np.zeros((NRANK * TPR, D), np.float16)
    buf[:z16_range.shape[0]] = z16_range
    return np.ascontiguousarray(
        buf.reshape(NRANK, TPR, D).transpose(1, 0, 2).reshape(TPR, NRANK * D))


def _wrap_idx(arr):
    """[n] int16 -> [128, n//16] wrapped (16-partition pattern, replicated)."""
    t = arr.reshape(-1, 16).T  # [16, n//16]
    return np.tile(t, (8, 1))


def _mlp_ref_f32(zs, zd, W1, b1, W2, b2, W3, b3):
    ef = np.concatenate([zs, zd], axis=1)
    h = np.maximum(ef @ W1 + b1, 0.0)
    h = np.maximum(h @ W2 + b2, 0.0)
    o = h @ W3 + b3
    return 1.0 / (1.0 + np.exp(-o[:, 0]))


def _pack_inputs(z, ei, W1, b1, W2, b2, W3, b3):
    """Shard edges by src range, bucket by dst range; build gather images."""
    z16 = z.astype(np.float16)
    zimg = [_z_image(z16[r * R:(r + 1) * R]) for r in range(NCLS)]
    w_common = {
        "w1s": np.ascontiguousarray(W1[:128].astype(np.float16)),
        "w1d": np.ascontiguousarray(W1[128:].astype(np.float16)),
        "w2a": np.ascontiguousarray(W2[:128].astype(np.float16)),
        "w2b": np.ascontiguousarray(W2[128:].astype(np.float16)),
        "w3v": _w3v(W3),
        "b1a": np.ascontiguousarray(b1[:128].reshape(128, 1)).astype(np.float32),
        "b1b": np.ascontiguousarray(b1[128:].reshape(128, 1)).astype(np.float32),
        "b2": np.ascontiguousarray(b2.reshape(128, 1)).astype(np.float32),
        "b3": np.full((128, 1), np.float32(b3.reshape(-1)[0])),
    }
    for k in range(NCLS):
        w_common[f"zd{k}"] = zimg[k]

    src_all, dst_all = ei[0], ei[1]
    core_of = src_all // R
    in_maps, metas = [], []
    for c in range(N_CORES):
        epos = np.nonzero(core_of == c)[0]
        src = src_all[epos]
        dst = dst_all[epos]
        cls = dst // R
        order = np.argsort(cls, kind="stable")
        counts = np.bincount(cls, minlength=NCLS)
        starts = np.zeros(NCLS + 1, np.int64)
        np.cumsum(counts, out=starts[1:])
        sidx = np.zeros(CAP, np.int16)
        didx = np.zeros(CAP, np.int16)
        kept, overflow = [], []
        for k in range(NCLS):
            seg = order[starts[k]:starts[k + 1]]
            if len(seg) > NI[k]:
                overflow.append(epos[seg[NI[k]:]])
                seg = seg[:NI[k]]
            n = len(seg)
            o = int(B_OFF[k]) * BLK
            sidx[o:o + n] = (src[seg] - c * R).astype(np.int16)
            didx[o:o + n] = (dst[seg] - k * R).astype(np.int16)
            kept.append(epos[seg])
        metas.append((kept, overflow))
        in_maps.append({
            **w_common,
            "zs": zimg[c],
            "sidx": np.ascontiguousarray(_wrap_idx(sidx)),
            "didx": np.ascontiguousarray(_wrap_idx(didx)),
        })
    return in_maps, metas


def _unpack_outputs(core_outs, metas, ei, z, W1, b1, W2, b2, W3, b3):
    E = ei.shape[1]
    out = np.empty(E, dtype=np.float32)
    for c in range(N_CORES):
        flat = np.asarray(core_outs[c], dtype=np.float32).reshape(CAP)
        kept, overflow = metas[c]
        for k in range(NCLS):
            pos = kept[k]
            o = int(B_OFF[k]) * BLK
            out[pos] = flat[o:o + len(pos)]
        for pos in overflow:
            # Host fallback for edges beyond static class capacity
            # (does not trigger for the benchmark dataset).
            out[pos] = _mlp_ref_f32(
                z[ei[0, pos]], z[ei[1, pos]], W1, b1, W2, b2, W3, b3)
    return out


def _run(z, edge_index, W1, b1, W2, b2, W3, b3, **spmd_kwargs):
    global _prog_cache
    z = np.asarray(z, dtype=np.float32)
    W1 = np.asarray(W1, dtype=np.float32)
    b1 = np.asarray(b1, dtype=np.float32)
    W2 = np.asarray(W2, dtype=np.float32)
    b2 = np.asarray(b2, dtype=np.float32)
    W3 = np.asarray(W3, dtype=np.float32)
    b3 = np.asarray(b3, dtype=np.float32)
    ei = np.asarray(edge_index).astype(np.int64)
    assert z.shape == (N_NODES, D) and ei.shape[0] == 2

    if _prog_cache is None:
        _prog_cache = _build_program()
    nc = _prog_cache

    in_maps, metas = _pack_inputs(z, ei, W1, b1, W2, b2, W3, b3)
    br = run_bass_kernel_spmd(nc, in_maps, list(range(N_CORES)), **spmd_kwargs)
    core_outs = [br.results[c]["out"] for c in range(N_CORES)]
    out = _unpack_outputs(core_outs, metas, ei, z, W1, b1, W2, b2, W3, b3)
    return out, br


def kernel(z, edge_index, W1, b1, W2, b2, W3, b3):
    out, _ = _run(z, edge_index, W1, b1, W2, b2, W3, b3)
    return out
